# revision 21
# baseline (speedup 1.0000x reference)
"""Block-causal (anti-causal: key-block >= query-block) multi-head attention
for Trainium2, run SPMD on 8 NeuronCores.

Problem (hardcoded): B=2, T=8, N=256 (L=2048), D=768, H=12, HD=64.
reference:
    qkv = x @ qkv_w.T + qkv_b ; split into q,k,v heads
    s   = (q @ k.T) / 8 ; mask: query in block ti attends keys in blocks tj >= ti
    p   = softmax(s) ; y = p @ v ; out = y @ proj_w.T + proj_b

Sharding: data-parallel over B (2) x tensor-parallel over heads (4 groups of
3 heads) = 8 cores. Each core computes, for its (batch, head-group):
  - QKV^T   = Wsel @ x^T  (transposed layouts; fp32r matmuls)
  - S^T     = K^T.T-chunks vs Q^T   (keys on partitions, queries on free dim)
  - P~      = exp(0.125 * S^T)      (no max-subtraction; logits are tiny)
  - U^T     = [V|1].T @ P~           (ones-column gives softmax denominator row)
  - O^T     = U^T * (1/den) broadcast
  - Z^T    += Wproj-slice @ O^T      (partial projection output)
Host sums the 4 head-group partials per batch and adds proj_b.
"""

import functools

import ml_dtypes
import numpy as np

import concourse.bass as bass
import concourse.bacc as bacc_mod
import concourse.mybir as mybir
import concourse.tile as tile
from concourse.bass import ts

F32 = mybir.dt.float32
F32R = mybir.dt.float32r
BF16 = mybir.dt.bfloat16

B, T, N, D = 2, 8, 256, 768
H, HD = 12, 64
L = T * N          # 2048
HPC = 3            # heads per core
NKC = L // 128     # 16 key chunks of 128
NQB = T            # 8 query blocks of 256
NDC = D // 128     # 6 contraction chunks
NNT = L // 512     # 4 tiles of 512 along L
SCALE = 1.0 / 8.0


def r(ap):
    """Matmul operands are float32r-typed tiles already; keep as-is."""
    return ap


def build_nc():
    nc = bacc_mod.Bacc()

    xT_d = nc.declare_dram_parameter("xT", [D, L], BF16, isOutput=False)
    wqkvT_d = nc.declare_dram_parameter("wqkvT", [D, 576], BF16, isOutput=False)
    bqkv_d = nc.declare_dram_parameter("bqkv", [1, 1152], BF16, isOutput=False)
    aux_d = nc.declare_dram_parameter("aux", [128, 80], BF16, isOutput=False)
    wprojT_d = nc.declare_dram_parameter("wprojT", [128, 1536], BF16, isOutput=False)
    zT_d = nc.declare_dram_parameter("zT", [D, L], F32, isOutput=True)

    with tile.TileContext(nc) as tc:
        with (
            tc.tile_pool(name="persist", bufs=1) as pp,
            tc.tile_pool(name="ptile", bufs=4) as ppool,
            tc.tile_pool(name="zbuf", bufs=3) as zpool,
            tc.tile_pool(name="psum_mm", bufs=4, space="PSUM") as pmm,
            tc.tile_pool(name="psum_ot", bufs=2, space="PSUM") as pot,
        ):
            # ---- persistent SBUF tensors ----
            wqkvT = pp.tile([128, NDC, 576], BF16, tag="wqkvT")
            bq = pp.tile([1, 1152], BF16, tag="bq")
            aux = pp.tile([128, 80], BF16, tag="aux")
            wprojT = pp.tile([128, 1536], BF16, tag="wprojT")
            # qkv-transposed activations: rows are head dims
            qt = pp.tile([128, L], BF16, tag="qt")      # [q_h0 | q_h1]
            kt = pp.tile([128, L], BF16, tag="kt")      # [k_h0 | k_h1]
            vt = pp.tile([128, L], BF16, tag="vt")      # [v_h0 | v_h1]
            qk2 = pp.tile([128, L], BF16, tag="qk2")    # [q_h2 | k_h2]
            kt2 = pp.tile([64, L], BF16, tag="kt2")     # k_h2 re-based to partition 0
            vt2 = pp.tile([64, L], BF16, tag="vt2")     # [v_h2]
            # natural-layout V per head, augmented with a ones column
            vn = [
                pp.tile([128, NKC, 65], BF16, tag=f"vn{h}", name=f"vn{h}")
                for h in range(HPC)
            ]
            # normalized attention outputs (transposed): rows are head dims
            otp = pp.tile([128, L], BF16, tag="otp")    # [o_h0 | o_h1]
            ots = pp.tile([64, L], BF16, tag="ots")     # [o_h2]

            for dc in range(NDC):
                nc.sync.dma_start(
                    out=wqkvT[:, dc, :], in_=wqkvT_d[ts(dc, 128), :]
                )
            nc.sync.dma_start(out=bq[:], in_=bqkv_d[:, :])
            nc.sync.dma_start(out=aux[:], in_=aux_d[:, :])
            nc.sync.dma_start(out=wprojT[:], in_=wprojT_d[:, :])
            # Pre-warm the exp activation table during the qkv phase.
            warm = zpool.tile([128, 32], F32, tag="warm")
            nc.vector.memset(warm[:], 0.0)
            nc.scalar.activation(warm[:], warm[:], mybir.ActivationFunctionType.Exp)

            # ---- phase 1: QKV^T = Wsel @ x^T, + bias ----
            # M-chunks of the 576 output dims (order fixed host-side):
            # 0:[q0|q1] 1:[k0|k1] 2:[v0|v1] 3:[q2|k2] 4:[v2] (64 rows)
            mc_dst = [qt, kt, vt, qk2, vt2]
            with tc.tile_pool(name="xT", bufs=1) as xp:
                xT = xp.tile([128, NDC, L], BF16, tag="xT")
                for nt in range(NNT):
                    for dc in range(NDC):
                        nc.sync.dma_start(
                            out=xT[:, dc, ts(nt, 512)],
                            in_=xT_d[ts(dc, 128), ts(nt, 512)],
                        )
                for nt in range(NNT):
                    for mc in range(5):
                        mrows = 64 if mc == 4 else 128
                        ps = pmm.tile([128, 512], F32, tag="mm")
                        for dc in range(NDC):
                            nc.tensor.matmul(
                                ps[0:mrows, :],
                                r(wqkvT[:, dc, mc * 128 : mc * 128 + mrows]),
                                r(xT[:, dc, ts(nt, 512)]),
                                start=(dc == 0),
                                stop=False,
                            )
                        # bias as a K=1 rank-1 update: psum += bias_row.T @ ones
                        nc.tensor.matmul(
                            ps[0:mrows, :],
                            bq[0:1, mc * 128 : mc * 128 + mrows],
                            bq[0:1, 640:1152],
                            start=False,
                            stop=True,
                        )
                        nc.vector.tensor_copy(
                            mc_dst[mc][0:mrows, ts(nt, 512)], ps[0:mrows, :]
                        )

            # ---- phase 2: V natural layout via PE transpose, + ones col ----
            vt_src = [vt[0:64, :], vt[64:128, :], vt2[0:64, :]]
            id_src = [aux[0:64, 0:64], aux[64:128, 0:64], aux[0:64, 0:64]]
            for h in range(HPC):
                nc.vector.tensor_copy(vn[h][:, :, 64], aux[:, 64:80])
                for kc in range(NKC):
                    tp = pmm.tile([128, 1024], BF16, tag="mm")
                    nc.tensor.transpose(
                        tp[:, 0:64], vt_src[h][:, ts(kc, 128)], id_src[h]
                    )
                    nc.vector.tensor_copy(vn[h][:, kc, 0:64], tp[:, 0:64])

            # ---- phase 3: attention per head ----
            # k_h2 sits at partitions 64:128 of qk2 while q_h2 is at 0:64; the
            # PE needs both matmul operands on the same partitions, so re-base
            # k_h2 with an SBUF->SBUF DMA (DMA is partition-agnostic).
            nc.gpsimd.dma_start(out=kt2[0:64, :], in_=qk2[64:128, :])
            qt_src = [qt[0:64, :], qt[64:128, :], qk2[0:64, :]]
            kt_src = [kt[0:64, :], kt[64:128, :], kt2[0:64, :]]
            ot_dst = [otp[0:64, :], otp[64:128, :], ots[0:64, :]]
            # Each (head, qblock) accumulator must own a full 2KB PSUM zero
            # region (start=True marks the whole region pending-zero), so
            # process 2 qblocks at a time with one bank each: qblock qb lives
            # at ot[:, (qb-qb0)*512 : (qb-qb0)*512+256]. Quarter granularity
            # keeps each O^T tile at 2 banks so two can be in flight and the
            # normalization chain overlaps the next quarter's matmuls.
            bcast = pp.tile([64, 512], F32, tag="bcast")
            den = pp.tile([1, 512], F32, tag="den")
            nc.vector.memset(bcast[:], 1.0)
            SHUF_ID0 = [0] * 32
            slotted = lambda ap: ap.rearrange("p (s c) -> p s c", c=512)[:, :, 0:256]

            def attn_step(h, kc, qq, ot):
                """S^T -> exp -> AV accumulate for one (head, key-chunk)."""
                qb0 = 2 * qq
                q_lo = qb0 * 256
                kb = kc // 2
                q_hi = (min(kb, qb0 + 1) + 1) * 256
                seg = q_hi - q_lo
                st = pmm.tile([128, 512], F32, tag="mm", name="st")
                nc.tensor.matmul(
                    st[:, 0:seg],
                    kt_src[h][:, ts(kc, 128)],
                    qt_src[h][:, q_lo:q_hi],
                    start=True,
                    stop=True,
                )
                pt = ppool.tile([128, 512], BF16, tag="pt", name="pt")
                nc.scalar.activation(
                    pt[:, 0:seg],
                    st[:, 0:seg],
                    mybir.ActivationFunctionType.Exp,
                    scale=SCALE,
                )
                for qb in range(qb0, qb0 + seg // 256):
                    qo = (qb - qb0) * 256
                    slot = qb - qb0
                    nc.tensor.matmul(
                        ot[0:65, slot * 512 : slot * 512 + 256],
                        vn[h][:, kc, :],
                        pt[:, qo : qo + 256],
                        start=(kc == 2 * qb),
                        stop=(kc == NKC - 1),
                        skip_group_check=True,
                    )

            def normalize(h, qq, ot):
                """inv = 1/den; broadcast across 64 partitions on the DVE
                (stream_shuffle); O^T = U^T * inv."""
                q_lo = qq * 512
                nc.vector.tensor_copy(
                    den[0:1, :].rearrange("p (s c) -> p s c", c=256),
                    slotted(ot[64:65, :]),
                )
                nc.vector.reciprocal_approx_fast(bcast[0:1, :], den[0:1, :])
                nc.vector.stream_shuffle(bcast[0:32, :], bcast[0:32, :], SHUF_ID0)
                nc.vector.stream_shuffle(bcast[32:64, :], bcast[0:32, :], SHUF_ID0)
                nc.vector.tensor_tensor(
                    out=ot_dst[h][:, q_lo : q_lo + 512].rearrange(
                        "p (s c) -> p s c", c=256
                    ),
                    in0=slotted(ot[0:64, :]),
                    in1=bcast[0:64, :].rearrange("p (s c) -> p s c", c=256),
                    op=mybir.AluOpType.mult,
                )

            # Quarter-outer / head-inner: h0 and h1 S^T matmuls sit on row
            # groups 0-1 / 2-3 and run concurrently on the PE; the per-quarter
            # projection injects dense K=128 N=512 matmuls that keep the HAM
            # clock warm and spreads the output DMA across the kernel.
            for qq in range(4):
                q_lo = qq * 512
                ot0 = pot.tile([128, 1024], F32, tag="ot", name="ot0")
                ot1 = pot.tile([128, 1024], F32, tag="ot", name="ot1")
                for kc in range(4 * qq, NKC):
                    attn_step(0, kc, qq, ot0)
                    attn_step(1, kc, qq, ot1)
                normalize(0, qq, ot0)
                normalize(1, qq, ot1)
                ot2 = pot.tile([128, 1024], F32, tag="ot", name="ot2")
                for kc in range(4 * qq, NKC):
                    attn_step(2, kc, qq, ot2)
                normalize(2, qq, ot2)
                # projection for this quarter's 512 query columns
                for mc in range(NDC):
                    ps = pmm.tile([128, 512], F32, tag="mm", name="psproj")
                    nc.tensor.matmul(
                        ps[:],
                        wprojT[:, ts(mc, 128)],
                        otp[:, q_lo : q_lo + 512],
                        start=True,
                        stop=False,
                    )
                    nc.tensor.matmul(
                        ps[:],
                        wprojT[0:64, 768 + mc * 128 : 768 + (mc + 1) * 128],
                        ots[0:64, q_lo : q_lo + 512],
                        start=False,
                        stop=True,
                    )
                    zb = zpool.tile([128, 512], F32, tag="zb", name="zb")
                    nc.scalar.copy(zb[:], ps[:])
                    nc.sync.dma_start(
                        out=zT_d[ts(mc, 128), q_lo : q_lo + 512], in_=zb[:]
                    )

    nc.compile()
    return nc


@functools.lru_cache(maxsize=1)
def get_nc():
    return build_nc()


def make_in_maps(x, qkv_w, qkv_b, proj_w):
    """Per-core host-side sharding/layout prep."""
    x = np.asarray(x, dtype=np.float32)
    qkv_w = np.asarray(qkv_w, dtype=np.float32)
    qkv_b = np.asarray(qkv_b, dtype=np.float32)
    proj_w = np.asarray(proj_w, dtype=np.float32)

    in_maps = []
    for c in range(8):
        b, g = divmod(c, 4)
        h0, h1, h2 = 3 * g, 3 * g + 1, 3 * g + 2

        def qrows(h):
            return slice(h * HD, (h + 1) * HD)

        def krows(h):
            return slice(D + h * HD, D + (h + 1) * HD)

        def vrows(h):
            return slice(2 * D + h * HD, 2 * D + (h + 1) * HD)

        order = [
            qrows(h0), qrows(h1), krows(h0), krows(h1), vrows(h0), vrows(h1),
            qrows(h2), krows(h2), vrows(h2),
        ]
        wsel = np.concatenate([qkv_w[s] for s in order], axis=0)      # (576, 768)
        bsel = np.concatenate([qkv_b[s] for s in order], axis=0)      # (576,)
        bpad = np.zeros(1152, np.float32)
        bpad[:576] = bsel
        bpad[640:] = 1.0
        wpp = np.concatenate(
            [proj_w[:, ts_np(h0)].T, proj_w[:, ts_np(h1)].T], axis=0
        )  # (128, 768)
        wps = np.concatenate(
            [proj_w[:, ts_np(h2)].T, np.zeros((64, D), np.float32)], axis=0
        )  # (128, 768)
        in_maps.append(
            {
                "xT": np.ascontiguousarray(x[b].reshape(L, D).T).astype(
                    ml_dtypes.bfloat16
                ),
                "wqkvT": np.ascontiguousarray(wsel.T).astype(ml_dtypes.bfloat16),
                "bqkv": np.ascontiguousarray(bpad.reshape(1, 1152)).astype(
                    ml_dtypes.bfloat16
                ),
                "aux": AUX.astype(ml_dtypes.bfloat16),
                "wprojT": np.ascontiguousarray(
                    np.concatenate([wpp, wps], axis=1)
                ).astype(ml_dtypes.bfloat16),
            }
        )
    return in_maps


AUX = np.concatenate(
    [
        np.concatenate([np.eye(64, dtype=np.float32)] * 2, axis=0),
        np.ones((128, 16), np.float32),
    ],
    axis=1,
)


def ts_np(h):
    return slice(h * HD, (h + 1) * HD)


def assemble_output(results, proj_b):
    proj_b = np.asarray(proj_b, dtype=np.float32)
    out = np.zeros((B, L, D), np.float32)
    for c in range(8):
        b = c // 4
        out[b] += results[c]["zT"].T
    out += proj_b[None, None, :]
    return out.reshape(B, T, N, D)


def _install_ntff_hook():
    """The container's antenv stub lacks axon_hooks; recreate it from the
    boot helper so trace=True can profile through libaxon_pjrt."""
    import sys
    import types

    try:
        from antenv.axon_hooks import get_axon_ntff_profile_hook  # noqa: F401

        return
    except ImportError:
        pass
    import antenv
    from trn_agent_boot.trn_boot import _ntff_profile_via_ctypes

    state = {"hook": _ntff_profile_via_ctypes("/opt/axon/libaxon_pjrt.so")}
    mod = types.ModuleType("antenv.axon_hooks")
    mod.set_axon_ntff_profile_hook = lambda h: state.__setitem__("hook", h)
    mod.get_axon_ntff_profile_hook = lambda: state["hook"]
    sys.modules["antenv.axon_hooks"] = mod
    antenv.axon_hooks = mod

    import concourse.bass_utils as bu

    orig_upload = bu.upload_artifacts

    def safe_upload(tmpdir):
        try:
            return orig_upload(tmpdir)
        except Exception:
            return tmpdir

    bu.upload_artifacts = safe_upload


def kernel_with_stats(x, qkv_w, qkv_b, proj_w, proj_b, trace=False):
    from concourse.bass_utils import run_bass_kernel_spmd

    if trace:
        _install_ntff_hook()
    nc = get_nc()
    in_maps = make_in_maps(x, qkv_w, qkv_b, proj_w)
    res = run_bass_kernel_spmd(nc, in_maps, list(range(8)), trace=trace)
    return assemble_output(res.results, proj_b), res


def kernel(x, qkv_w, qkv_b, proj_w, proj_b):
    out, _ = kernel_with_stats(x, qkv_w, qkv_b, proj_w, proj_b)
    return out
